# revision 20
# baseline (speedup 1.0000x reference)
"""Pairwise ranking loss kernel for Trainium2 (8 NeuronCores, data-parallel over batch).

reference semantics (per sample, N=512):
    m[j,k]   = mask[j]*mask[k]
    s[j,k]   = sigmoid(5*(o[j]-o[k])) * m
    t1[j,k]  = (1 if t[j]>t[k] else 0 if t[j]<t[k] else 0.5) * m
    hm       = (t1 != 0.5)
    loss     = (s*hm - t1*hm)^2 * m

For binary mask this reduces to
    loss[j,k] = sigmoid(-5*sign(dt)*(o[j]-o[k]))^2   if t[j]!=t[k] and m=1
              = 0                                    otherwise
which we fold into a single bf16 matmul producing
    W[j,k] = -5*sign(dt)*(o[j]-o[k]) - C*[t[j]==t[k]] - C*(1-m[j]) - C*(1-m[k])
followed by loss = sigmoid(W)^2 on-chip (ACT sigmoid + DVE square).

The matmul uses a one-hot expansion over the 10 possible integer target
values; fp32 o-values are split into three exact bf16 terms (h+l+q) so
every stored bf16 entry is exact and the fp32 PSUM accumulation
reconstructs W to ~1e-6 absolute.

Layout: two samples share the 128 SBUF partitions (even sample in rows
0-63, odd in rows 64-127 — matmul requires lhsT/rhs base partitions to
match). One packed [128, 4096] bf16 input per core: cols 0-2047 hold the
stationary operands for sample-pairs 0-3, cols 2048-4095 the moving
operands. Loaded pair-0-first so compute starts ~1us after the preamble;
the first sample is processed at [128,512] chunk granularity so the
output-DMA stream (the 8 MB/core roofline term) starts as early as
possible.
"""

import numpy as np
import ml_dtypes

B = 64          # batch
N = 512         # items per sample
NCORES = 8
S = B // NCORES  # samples per core
NV = 10          # target values 0..9
KROWS = 64       # contraction rows (62 used + 2 pad)
C_BIG = 20480.0  # = 5*4096; exact in bf16; sigmoid(-20480) == 0 in fp32

_BF16 = ml_dtypes.bfloat16

_PROG = None  # cached program — input-independent

LAST_RESULTS = None  # BassKernelResults of the most recent run (for test.py)


def _bf16_split3(x):
    """Split fp32 array into h+l+q, each exactly representable in bf16,
    with x - (h+l+q) ~ 2^-24 relative."""
    h = x.astype(_BF16).astype(np.float32)
    r = x - h
    l = r.astype(_BF16).astype(np.float32)
    q = (r - l).astype(_BF16).astype(np.float32)
    return h, l, q


def _prep_operands(output, target, mask):
    """Build the packed [128, 2*S*N/2... ] = [128, 4096] bf16 input per core.

    Row layout: rows 0-63 = even sample of a pair, rows 64-127 = odd.
    Col layout: p*N+j for pair p in [0,4) on the left half (stationary),
    2048 + p*N+j on the right half (moving)."""
    o = np.asarray(output, dtype=np.float32)
    t = np.asarray(target).astype(np.int32)
    m = np.asarray(mask, dtype=np.float32)

    h, l, q = _bf16_split3(o)                      # [B, N] each
    vals = np.arange(NV, dtype=np.int32)
    oh = (t[:, None, :] == vals[None, :, None])    # [B, NV, N] bool
    ohf = oh.astype(np.float32)
    sgn = np.sign(vals[None, :, None] - t[:, None, :]).astype(np.float32)

    lhsT = np.zeros((B, KROWS, N), np.float32)
    lhsT[:, 0:10] = ohf * h[:, None, :]
    lhsT[:, 10:20] = ohf * l[:, None, :]
    lhsT[:, 20:30] = ohf * q[:, None, :]
    lhsT[:, 30:40] = 5.0 * ohf
    lhsT[:, 40:50] = 5.0 * ohf
    lhsT[:, 50:60] = 5.0 * ohf
    lhsT[:, 60] = -C_BIG * (1.0 - m)
    lhsT[:, 61] = 1.0

    rhs = np.zeros((B, KROWS, N), np.float32)
    rhs[:, 0:10] = -5.0 * sgn
    rhs[:, 10:20] = -5.0 * sgn
    rhs[:, 20:30] = -5.0 * sgn
    rhs[:, 30:40] = np.where(oh, np.float32(-4096.0), h[:, None, :] * sgn)
    rhs[:, 40:50] = l[:, None, :] * sgn
    rhs[:, 50:60] = q[:, None, :] * sgn
    rhs[:, 60] = 1.0
    rhs[:, 61] = -C_BIG * (1.0 - m)

    npairs = S // 2
    packed = []
    for i in range(NCORES):
        arr = np.zeros((128, 2 * npairs * N), np.float32)
        for p in range(npairs):
            for r in range(2):
                b = i * S + 2 * p + r
                arr[64 * r:64 * (r + 1), p * N:(p + 1) * N] = lhsT[b]
                arr[64 * r:64 * (r + 1), npairs * N + p * N:
                    npairs * N + (p + 1) * N] = rhs[b]
        packed.append(arr.astype(_BF16))
    return packed


def _build_program():
    from contextlib import ExitStack

    import concourse.bacc as bacc
    from concourse import mybir

    nc = bacc.Bacc(None, target_bir_lowering=False)
    HALF = (S // 2) * N  # 2048
    packed = nc.declare_dram_parameter("packed", [128, 2 * HALF],
                                       mybir.dt.bfloat16, isOutput=False)
    loss = nc.declare_dram_parameter("loss", [S * N, N], mybir.dt.float32,
                                     isOutput=True)

    CH = N // 128  # row-chunks per sample (4)
    f32 = mybir.dt.float32

    # elementwise schedule: (sample, chunk-group-start, group-width-chunks,
    # square-engine). Early samples run fine-grained so the output-DMA
    # stream starts early; the first two squares run on ACT right after
    # their sigmoid (no cross-engine hop). Later samples run full-width
    # (lowest op overhead; DVE squares).
    OPS = []
    for s in range(S):
        grp = 1 if s == 0 else (2 if s in (1, 2) else CH)
        for g in range(0, CH, grp):
            sq = "act" if s == S - 1 else "dve"
            OPS.append((s, g, grp, sq))
    NOPS = len(OPS)
    LAST_OP = {s: max(i for i, o in enumerate(OPS) if o[0] == s)
               for s in range(S)}
    # running per-engine square counts (1-based at op a)
    NDVE, NASQ = [], []
    nd = na = 0
    for (_, _, _, sq) in OPS:
        if sq == "dve":
            nd += 1
        else:
            na += 1
        NDVE.append(nd)
        NASQ.append(na)
    NBUF = 4  # st/qt ring depth
    WMAX = CH * N

    with ExitStack() as ctx:
        allin = ctx.enter_context(nc.sbuf_tensor("allin", [128, 2 * HALF],
                                                 mybir.dt.bfloat16))
        psum = [ctx.enter_context(nc.psum_tensor(f"psum{i}", [128, CH * N],
                                                 f32))
                for i in range(2)]
        st = [ctx.enter_context(nc.sbuf_tensor(f"st{i}", [128, WMAX], f32))
              for i in range(NBUF)]
        qt = [ctx.enter_context(nc.sbuf_tensor(f"qt{i}", [128, WMAX], f32))
              for i in range(NBUF)]
        warm = ctx.enter_context(nc.sbuf_tensor("warm", [64, 128],
                                                mybir.dt.bfloat16))
        s_wm = ctx.enter_context(nc.semaphore("s_wm"))
        s_in0 = ctx.enter_context(nc.semaphore("s_in0"))
        s_in1 = ctx.enter_context(nc.semaphore("s_in1"))
        s_pe = ctx.enter_context(nc.semaphore("s_pe"))
        s_act = ctx.enter_context(nc.semaphore("s_act"))
        s_asq = ctx.enter_context(nc.semaphore("s_asq"))
        s_dve = ctx.enter_context(nc.semaphore("s_dve"))
        s_q = [ctx.enter_context(nc.semaphore(f"s_q{i}"))
               for i in range(NBUF)]
        block = ctx.enter_context(nc.Block())

        def lhs_ap(s, c):
            p, r = s // 2, s % 2
            return allin[64 * r:64 * r + KROWS,
                         p * N + c * 128: p * N + (c + 1) * 128]

        def rhs_ap(s):
            p, r = s // 2, s % 2
            return allin[64 * r:64 * r + KROWS, HALF + p * N: HALF + (p + 1) * N]

        def wait_square_done(eng, a):
            """wait until the square of op a has completed"""
            if OPS[a][3] == "dve":
                eng.wait_ge(s_dve, NDVE[a])
            else:
                eng.wait_ge(s_asq, NASQ[a])

        @block.gpsimd
        def _(gpsimd):
            nc.gpsimd.memset(warm[:], 0.0).then_inc(s_wm, 1)

        @block.sync
        def _(sync):
            # input: sample-pair 0 first, then the rest (full 128-partition BW)
            src = packed[:].rearrange("p (h c) -> p h c", h=2)
            dst = allin[:].rearrange("p (h c) -> p h c", h=2)
            sync.dma_start(out=dst[:, :, 0:N],
                           in_=src[:, :, 0:N]).then_inc(s_in0, 16)
            sync.dma_start(out=dst[:, :, N:HALF],
                           in_=src[:, :, N:HALF]).then_inc(s_in1, 16)
            for a, (s, g, grp, sq) in enumerate(OPS):
                w = grp * N
                wait_square_done(sync, a)
                out_view = loss[s * N:(s + 1) * N, :].rearrange(
                    "(c p) k -> p c k", p=128)
                sync.dma_start(
                    out=out_view[:, g:g + grp, :],
                    in_=qt[a % NBUF][:, 0:w].rearrange(
                        "p (c k) -> p c k", k=N)).then_inc(s_q[a % NBUF], 16)
            for i in range(NBUF):
                ndma = len([1 for a in range(NOPS) if a % NBUF == i])
                sync.wait_ge(s_q[i], 16 * ndma)

        @block.tensor
        def _(tensor):
            # warm the PE HAM clock-gate while the input DMA is in flight:
            # ~24 back-to-back LDWEIGHTS keep the array busy so the real
            # matmuls run at 2.4 GHz instead of the cold 1.2 GHz
            tensor.wait_ge(s_wm, 1)
            for _ in range(24):
                nc.tensor.ldweights(warm[:])
            tensor.wait_ge(s_in0, 16)         # pair 0 resident
            for s in range(S):
                if s == 2:
                    tensor.wait_ge(s_in1, 16)  # rest resident
                if s >= 2:
                    # psum[s%2] free once sample s-2's last ACT read it
                    tensor.wait_ge(s_act, LAST_OP[s - 2] + 1)
                for c in range(CH):
                    nc.tensor.matmul(psum[s % 2][:, c * N:(c + 1) * N],
                                     lhs_ap(s, c), rhs_ap(s),
                                     start=True, stop=True).then_inc(s_pe, 1)

        @block.scalar
        def _(scalar):
            for a, (s, g, grp, sq) in enumerate(OPS):
                w = grp * N
                # matmuls for chunks [g, g+grp) of sample s done
                scalar.wait_ge(s_pe, CH * s + g + grp)
                if a >= NBUF:
                    # st[a%NBUF] free once the square of op a-NBUF read it
                    wait_square_done(scalar, a - NBUF)
                nc.scalar.activation(
                    out=st[a % NBUF][:, 0:w],
                    in_=psum[s % 2][:, g * N:g * N + w],
                    func=mybir.ActivationFunctionType.Sigmoid,
                ).then_inc(s_act, 1)
                if sq == "act":
                    # own sigmoid may still be in the ACT pipeline
                    scalar.wait_ge(s_act, a + 1)
                    if a >= NBUF:
                        scalar.wait_ge(s_q[a % NBUF], 16 * (a // NBUF))
                    nc.scalar.square(
                        out=qt[a % NBUF][:, 0:w],
                        in_=st[a % NBUF][:, 0:w]).then_inc(s_asq, 1)

        @block.vector
        def _(vector):
            for a, (s, g, grp, sq) in enumerate(OPS):
                if sq != "dve":
                    continue
                w = grp * N
                vector.wait_ge(s_act, a + 1)
                if a >= NBUF:
                    # qt[a%NBUF] free once out-DMA a-NBUF completed
                    # (same-slot DMAs are chain-ordered, so per-slot
                    # counting is race-free)
                    vector.wait_ge(s_q[a % NBUF], 16 * (a // NBUF))
                nc.vector.tensor_mul(qt[a % NBUF][:, 0:w],
                                     st[a % NBUF][:, 0:w],
                                     st[a % NBUF][:, 0:w]).then_inc(s_dve, 1)

    nc.compile()
    return nc


def _get_program():
    global _PROG
    if _PROG is None:
        _PROG = _build_program()
    return _PROG


def kernel(output, target, mask):
    global LAST_RESULTS
    from concourse.bass_utils import run_bass_kernel_spmd

    packed = _prep_operands(output, target, mask)
    nc = _get_program()
    in_maps = [{"packed": packed[i]} for i in range(NCORES)]
    res = run_bass_kernel_spmd(nc, in_maps, core_ids=list(range(NCORES)))
    LAST_RESULTS = res
    out = np.concatenate(
        [np.asarray(res.results[i]["loss"]).reshape(S, N, N)
         for i in range(NCORES)], axis=0)
    return out.astype(np.float32)


# revision 21
# speedup vs baseline: 1.0677x; 1.0677x over previous
"""Pairwise ranking loss kernel for Trainium2 (8 NeuronCores, data-parallel over batch).

reference semantics (per sample, N=512):
    m[j,k]   = mask[j]*mask[k]
    s[j,k]   = sigmoid(5*(o[j]-o[k])) * m
    t1[j,k]  = (1 if t[j]>t[k] else 0 if t[j]<t[k] else 0.5) * m
    hm       = (t1 != 0.5)
    loss     = (s*hm - t1*hm)^2 * m

For binary mask this reduces to
    loss[j,k] = sigmoid(-5*sign(dt)*(o[j]-o[k]))^2   if t[j]!=t[k] and m=1
              = 0                                    otherwise
which we fold into a single bf16 matmul producing
    W[j,k] = -5*sign(dt)*(o[j]-o[k]) - C*[t[j]==t[k]] - C*(1-m[j]) - C*(1-m[k])
followed by loss = sigmoid(W)^2 on-chip (ACT sigmoid + DVE square).

The matmul uses a one-hot expansion over the 10 possible integer target
values; fp32 o-values are split into three exact bf16 terms (h+l+q) so
every stored bf16 entry is exact and the fp32 PSUM accumulation
reconstructs W to ~1e-6 absolute.

Layout: two samples share the 128 SBUF partitions (even sample in rows
0-63, odd in rows 64-127 — matmul requires lhsT/rhs base partitions to
match). One packed [128, 4096] bf16 input per core: cols 0-2047 hold the
stationary operands for sample-pairs 0-3, cols 2048-4095 the moving
operands. Loaded pair-0-first so compute starts ~1us after the preamble.

The device program is raw Bass (per-engine instruction streams with
manual semaphores, no Tile scheduler — avoids Tile's multi-us exit
barrier). Pipeline per sample: 4 matmuls (PE) -> sigmoid (ACT,
PSUM->SBUF) -> square (DVE) -> DMA out (sync/HWDGE). The kernel is
bound by the 8 MB/core output write (~23 us at ~360 GB/s HBM), so the
first samples run at fine chunk granularity to start the output stream
early, the last sample's square runs on ACT so DVE isn't the tail, and
PSUM ping-pongs between two 4-bank tiles.
"""

import numpy as np
import ml_dtypes

B = 64          # batch
N = 512         # items per sample
NCORES = 8
S = B // NCORES  # samples per core
NV = 10          # target values 0..9
KROWS = 64       # contraction rows (62 used + 2 pad)
C_BIG = 20480.0  # = 5*4096; exact in bf16; sigmoid(-20480) == 0 in fp32

_BF16 = ml_dtypes.bfloat16

_PROG = None  # cached program — input-independent

LAST_RESULTS = None  # BassKernelResults of the most recent run (for test.py)


def _bf16_split3(x):
    """Split fp32 array into h+l+q, each exactly representable in bf16,
    with x - (h+l+q) ~ 2^-24 relative."""
    h = x.astype(_BF16).astype(np.float32)
    r = x - h
    l = r.astype(_BF16).astype(np.float32)
    q = (r - l).astype(_BF16).astype(np.float32)
    return h, l, q


def _prep_operands(output, target, mask):
    """Build the packed [128, 2*S*N/2... ] = [128, 4096] bf16 input per core.

    Row layout: rows 0-63 = even sample of a pair, rows 64-127 = odd.
    Col layout: p*N+j for pair p in [0,4) on the left half (stationary),
    2048 + p*N+j on the right half (moving)."""
    o = np.asarray(output, dtype=np.float32)
    t = np.asarray(target).astype(np.int32)
    m = np.asarray(mask, dtype=np.float32)

    h, l, q = _bf16_split3(o)                      # [B, N] each
    vals = np.arange(NV, dtype=np.int32)
    oh = (t[:, None, :] == vals[None, :, None])    # [B, NV, N] bool
    ohf = oh.astype(np.float32)
    sgn = np.sign(vals[None, :, None] - t[:, None, :]).astype(np.float32)

    lhsT = np.zeros((B, KROWS, N), np.float32)
    lhsT[:, 0:10] = ohf * h[:, None, :]
    lhsT[:, 10:20] = ohf * l[:, None, :]
    lhsT[:, 20:30] = ohf * q[:, None, :]
    lhsT[:, 30:40] = 5.0 * ohf
    lhsT[:, 40:50] = 5.0 * ohf
    lhsT[:, 50:60] = 5.0 * ohf
    lhsT[:, 60] = -C_BIG * (1.0 - m)
    lhsT[:, 61] = 1.0

    rhs = np.zeros((B, KROWS, N), np.float32)
    rhs[:, 0:10] = -5.0 * sgn
    rhs[:, 10:20] = -5.0 * sgn
    rhs[:, 20:30] = -5.0 * sgn
    rhs[:, 30:40] = np.where(oh, np.float32(-4096.0), h[:, None, :] * sgn)
    rhs[:, 40:50] = l[:, None, :] * sgn
    rhs[:, 50:60] = q[:, None, :] * sgn
    rhs[:, 60] = 1.0
    rhs[:, 61] = -C_BIG * (1.0 - m)

    npairs = S // 2
    packed = []
    for i in range(NCORES):
        arr = np.zeros((128, 2 * npairs * N), np.float32)
        for p in range(npairs):
            for r in range(2):
                b = i * S + 2 * p + r
                arr[64 * r:64 * (r + 1), p * N:(p + 1) * N] = lhsT[b]
                arr[64 * r:64 * (r + 1), npairs * N + p * N:
                    npairs * N + (p + 1) * N] = rhs[b]
        packed.append(arr.astype(_BF16))
    return packed


def _build_program():
    from contextlib import ExitStack

    import concourse.bacc as bacc
    from concourse import mybir

    nc = bacc.Bacc(None, target_bir_lowering=False)
    HALF = (S // 2) * N  # 2048
    packed = nc.declare_dram_parameter("packed", [128, 2 * HALF],
                                       mybir.dt.bfloat16, isOutput=False)
    loss = nc.declare_dram_parameter("loss", [S * N, N], mybir.dt.float32,
                                     isOutput=True)

    CH = N // 128  # row-chunks per sample (4)
    f32 = mybir.dt.float32

    # elementwise schedule: (sample, chunk-group-start, group-width-chunks,
    # square-engine). Early samples run fine-grained so the output-DMA
    # stream starts early; the first two squares run on ACT right after
    # their sigmoid (no cross-engine hop). Later samples run full-width
    # (lowest op overhead; DVE squares).
    OPS = []
    for s in range(S):
        grp = 1 if s == 0 else (2 if s in (1, 2) else CH)
        for g in range(0, CH, grp):
            sq = "act" if s == S - 1 else "dve"
            OPS.append((s, g, grp, sq))
    NOPS = len(OPS)
    LAST_OP = {s: max(i for i, o in enumerate(OPS) if o[0] == s)
               for s in range(S)}
    # running per-engine square counts (1-based at op a)
    NDVE, NASQ = [], []
    nd = na = 0
    for (_, _, _, sq) in OPS:
        if sq == "dve":
            nd += 1
        else:
            na += 1
        NDVE.append(nd)
        NASQ.append(na)
    NBUF = 4  # st/qt ring depth
    WMAX = CH * N

    with ExitStack() as ctx:
        allin = ctx.enter_context(nc.sbuf_tensor("allin", [128, 2 * HALF],
                                                 mybir.dt.bfloat16))
        psum = [ctx.enter_context(nc.psum_tensor(f"psum{i}", [128, CH * N],
                                                 f32))
                for i in range(2)]
        st = [ctx.enter_context(nc.sbuf_tensor(f"st{i}", [128, WMAX], f32))
              for i in range(NBUF)]
        qt = [ctx.enter_context(nc.sbuf_tensor(f"qt{i}", [128, WMAX], f32))
              for i in range(NBUF)]
        warm = ctx.enter_context(nc.sbuf_tensor("warm", [64, 128],
                                                mybir.dt.bfloat16))
        s_wm = ctx.enter_context(nc.semaphore("s_wm"))
        s_in0 = ctx.enter_context(nc.semaphore("s_in0"))
        s_in1 = ctx.enter_context(nc.semaphore("s_in1"))
        s_pe = ctx.enter_context(nc.semaphore("s_pe"))
        s_act = ctx.enter_context(nc.semaphore("s_act"))
        s_asq = ctx.enter_context(nc.semaphore("s_asq"))
        s_dve = ctx.enter_context(nc.semaphore("s_dve"))
        s_q = [ctx.enter_context(nc.semaphore(f"s_q{i}"))
               for i in range(NBUF)]
        block = ctx.enter_context(nc.Block())

        def lhs_ap(s, c):
            p, r = s // 2, s % 2
            return allin[64 * r:64 * r + KROWS,
                         p * N + c * 128: p * N + (c + 1) * 128]

        def rhs_ap(s):
            p, r = s // 2, s % 2
            return allin[64 * r:64 * r + KROWS, HALF + p * N: HALF + (p + 1) * N]

        def wait_square_done(eng, a):
            """wait until the square of op a has completed"""
            if OPS[a][3] == "dve":
                eng.wait_ge(s_dve, NDVE[a])
            else:
                eng.wait_ge(s_asq, NASQ[a])

        @block.gpsimd
        def _(gpsimd):
            nc.gpsimd.memset(warm[:], 0.0).then_inc(s_wm, 1)

        @block.sync
        def _(sync):
            # input: sample-pair 0 first, then the rest (full 128-partition BW)
            src = packed[:].rearrange("p (h c) -> p h c", h=2)
            dst = allin[:].rearrange("p (h c) -> p h c", h=2)
            sync.dma_start(out=dst[:, :, 0:N],
                           in_=src[:, :, 0:N]).then_inc(s_in0, 16)
            sync.dma_start(out=dst[:, :, N:HALF],
                           in_=src[:, :, N:HALF]).then_inc(s_in1, 16)
            for a, (s, g, grp, sq) in enumerate(OPS):
                w = grp * N
                wait_square_done(sync, a)
                out_view = loss[s * N:(s + 1) * N, :].rearrange(
                    "(c p) k -> p c k", p=128)
                sync.dma_start(
                    out=out_view[:, g:g + grp, :],
                    in_=qt[a % NBUF][:, 0:w].rearrange(
                        "p (c k) -> p c k", k=N)).then_inc(s_q[a % NBUF], 16)
            for i in range(NBUF):
                ndma = len([1 for a in range(NOPS) if a % NBUF == i])
                sync.wait_ge(s_q[i], 16 * ndma)

        @block.tensor
        def _(tensor):
            # warm the PE HAM clock-gate while the input DMA is in flight:
            # ~24 back-to-back LDWEIGHTS keep the array busy so the real
            # matmuls run at 2.4 GHz instead of the cold 1.2 GHz
            tensor.wait_ge(s_wm, 1)
            for _ in range(24):
                nc.tensor.ldweights(warm[:])
            tensor.wait_ge(s_in0, 16)         # pair 0 resident
            for s in range(S):
                if s == 2:
                    tensor.wait_ge(s_in1, 16)  # rest resident
                if s >= 2:
                    # psum[s%2] free once sample s-2's last ACT read it
                    tensor.wait_ge(s_act, LAST_OP[s - 2] + 1)
                for c in range(CH):
                    nc.tensor.matmul(psum[s % 2][:, c * N:(c + 1) * N],
                                     lhs_ap(s, c), rhs_ap(s),
                                     start=True, stop=True).then_inc(s_pe, 1)

        @block.scalar
        def _(scalar):
            for a, (s, g, grp, sq) in enumerate(OPS):
                w = grp * N
                # matmuls for chunks [g, g+grp) of sample s done
                scalar.wait_ge(s_pe, CH * s + g + grp)
                if a >= NBUF:
                    # st[a%NBUF] free once the square of op a-NBUF read it
                    wait_square_done(scalar, a - NBUF)
                nc.scalar.activation(
                    out=st[a % NBUF][:, 0:w],
                    in_=psum[s % 2][:, g * N:g * N + w],
                    func=mybir.ActivationFunctionType.Sigmoid,
                ).then_inc(s_act, 1)
                if sq == "act":
                    # own sigmoid may still be in the ACT pipeline
                    scalar.wait_ge(s_act, a + 1)
                    if a >= NBUF:
                        scalar.wait_ge(s_q[a % NBUF], 16 * (a // NBUF))
                    nc.scalar.square(
                        out=qt[a % NBUF][:, 0:w],
                        in_=st[a % NBUF][:, 0:w]).then_inc(s_asq, 1)

        @block.vector
        def _(vector):
            for a, (s, g, grp, sq) in enumerate(OPS):
                if sq != "dve":
                    continue
                w = grp * N
                vector.wait_ge(s_act, a + 1)
                if a >= NBUF:
                    # qt[a%NBUF] free once out-DMA a-NBUF completed
                    # (same-slot DMAs are chain-ordered, so per-slot
                    # counting is race-free)
                    vector.wait_ge(s_q[a % NBUF], 16 * (a // NBUF))
                nc.vector.tensor_mul(qt[a % NBUF][:, 0:w],
                                     st[a % NBUF][:, 0:w],
                                     st[a % NBUF][:, 0:w]).then_inc(s_dve, 1)

    nc.compile()
    return nc


def _get_program():
    global _PROG
    if _PROG is None:
        _PROG = _build_program()
    return _PROG


def kernel(output, target, mask):
    global LAST_RESULTS
    from concourse.bass_utils import run_bass_kernel_spmd

    packed = _prep_operands(output, target, mask)
    nc = _get_program()
    in_maps = [{"packed": packed[i]} for i in range(NCORES)]
    res = run_bass_kernel_spmd(nc, in_maps, core_ids=list(range(NCORES)))
    LAST_RESULTS = res
    out = np.concatenate(
        [np.asarray(res.results[i]["loss"]).reshape(S, N, N)
         for i in range(NCORES)], axis=0)
    return out.astype(np.float32)
